# revision 1
# baseline (speedup 1.0000x reference)
"""Dice loss (sigmoid + per-sample weighted sums) on 8 Trainium2 NeuronCores.

Data-parallel: the flattened per-sample element axis (192^3 = 7,077,888) is
sharded contiguously across 8 cores (884,736 elements = [128 x 6912] each).
Each core computes per-partition partial sums of sigmoid(pred), of
sigmoid(pred)*target, and of target for each of the 3 samples; the host sums
the partials and finishes the dice formula (per the data-parallel hint).

Per-core pipeline (memory-bound; ~21.2 MB HBM traffic/core):
  per chunk: pred DMA on the sync HWDGE ring, target DMA on the scalar HWDGE
  ring (splitting issue across both rings measured faster on HW);
  ScalarE sigmoid with fused per-partition accumulate (sum p);
  VectorE scalar_tensor_tensor p*t with fused accumulate (sum p*t);
  sum t alternates between VectorE tensor_reduce and ScalarE copy+accumulate.
  All partials land in one shared SBUF stats tile -> single output DMA.
  Samples 0-1 use 1728-wide chunks (fewer DMAs); sample 2 uses 864-wide
  chunks so the pipeline tail after the last DMA is shorter.
"""

import numpy as np

import concourse.bacc as bacc
import concourse.tile as tile
from concourse import mybir
from concourse.bass_utils import run_bass_kernel_spmd
from concourse.vector_clock import ScopedClock


class _LeanTileContext(tile.TileContext):
    """Tile exit for single-TileContext kernels, three changes vs stock:

    1. The final output DMA is issued here, between the drain and the barrier,
       on a non-Tile semaphore — its ~1.5 us HBM write receipt then overlaps
       the exit barrier and the semaphore clears instead of serializing before
       them. gpsimd waits the receipt last and resets the semaphore so
       re-execution of the loaded NEFF sees a clean state.
    2. The trailing all-engine barrier is dropped (it only fences semaphore
       reuse by a subsequent TileContext, which this kernel doesn't have).
    3. The unused PE engine is excluded from the pre-clear barrier.

    NRT re-executes a NEFF only after every engine halted, and gpsimd halts
    after the clears + receipt wait, so re-execution is safe. Validated on HW
    over 10 consecutive dispatches of one loaded executable."""

    final_dma = None  # (out_dram_ap, stats_tile_ap) set by _build

    def _drain_and_barrier(self, tick_clock, wait_clock):
        nc = self.nc
        drain_inst = nc.sync.drain()
        wait_clock.add_sem_waits(
            drain_inst.ins, ScopedClock({None: tick_clock.global_clock})
        )
        out_sem = None
        if self.final_dma is not None:
            out_ap, in_ap = self.final_dma
            if self.is_my_tile(in_ap.tensor):
                in_ap.tensor = in_ap.tensor.concrete_tensor()
            out_sem = nc.alloc_semaphore("final_out_dma_sem")
            nc.sync.dma_start(out=out_ap, in_=in_ap).then_inc(out_sem, 16)
        nc.multi_engine_barrier(
            [
                mybir.EngineType.SP,
                mybir.EngineType.Activation,
                mybir.EngineType.DVE,
                mybir.EngineType.Pool,
            ]
        )
        popped = nc._tile_sem_poison_stack.pop()
        assert popped is self._sem_poison
        nc.clear_and_free_semaphores(list(self.sems.allocated().values()))
        if out_sem is not None:
            nc.gpsimd.wait_ge(out_sem, 16)
            nc.gpsimd.sem_clear(out_sem)

B = 3                 # batch (samples)
N_CORES = 8
D = 192
N = D * D * D         # 7,077,888 elements per sample
SHARD = N // N_CORES  # 884,736 per core per sample
P = 128               # SBUF partitions
F = SHARD // P        # 6912 free elements per partition

# chunk plan per sample (each list must sum to F); uniform 1728 measured
# ~0.9 us/iter faster than a 1728/864 hybrid in an interleaved HW A/B
PLANS = [[1728] * 4, [1728] * 4, [1728] * 4]
NCOLS = sum(len(p) for p in PLANS)          # stat columns per quantity (16)
SAMPLE_COL_OFFSETS = np.cumsum([0] + [len(p) for p in PLANS])  # [0, 4, 8, 16]
MAXC = max(max(p) for p in PLANS)
FP32 = mybir.dt.float32
BF16 = mybir.dt.bfloat16

_nc_cache = None


def _build(repeat=1):
    nc = bacc.Bacc("TRN2")
    pred = nc.dram_tensor("pred", [B, P, F], FP32, kind="ExternalInput")
    targ = nc.dram_tensor("target", [B, P, F], FP32, kind="ExternalInput")
    # out[:, q*NCOLS + k]: q=0 -> sum sigmoid(p), q=1 -> sum p*t, q=2 -> sum t
    out = nc.dram_tensor("out", [P, 3 * NCOLS], FP32, kind="ExternalOutput")

    with _LeanTileContext(nc) as tc:
        with (
            tc.tile_pool(name="io", bufs=6) as io,
            tc.tile_pool(name="tmp", bufs=3) as tmp,
            tc.tile_pool(name="stats", bufs=1) as stats,
        ):
            st = stats.tile([P, 3 * NCOLS], FP32, tag="st")
            st_p = st[:, 0:NCOLS]
            st_pt = st[:, NCOLS : 2 * NCOLS]
            st_t = st[:, 2 * NCOLS : 3 * NCOLS]
            for _ in range(repeat):
                k = 0
                for b, plan in enumerate(PLANS):
                    off = 0
                    for ch in plan:
                        p_in = io.tile([P, MAXC], FP32, tag="p_in")
                        t_in = io.tile([P, MAXC], FP32, tag="t_in")
                        cols = slice(off, off + ch)
                        # split input DMA issue across both HWDGE rings
                        nc.sync.dma_start(out=p_in[:, :ch], in_=pred[b, :, cols])
                        nc.scalar.dma_start(out=t_in[:, :ch], in_=targ[b, :, cols])

                        sig = tmp.tile([P, MAXC], FP32, tag="sig")
                        nc.scalar.activation(
                            sig[:, :ch],
                            p_in[:, :ch],
                            mybir.ActivationFunctionType.Sigmoid,
                            accum_out=st_p[:, k : k + 1],
                        )
                        # prod/tcopy are discarded side-outputs of the fused
                        # accumulate ops: bf16 halves their SBUF write traffic
                        # (contending with the DMA input stream) while the
                        # accumulation itself stays fp32 (HW-verified 1e-6).
                        prod = tmp.tile([P, MAXC], BF16, tag="prod")
                        nc.vector.scalar_tensor_tensor(
                            out=prod[:, :ch],
                            in0=sig[:, :ch],
                            scalar=0.0,
                            in1=t_in[:, :ch],
                            op0=mybir.AluOpType.bypass,
                            op1=mybir.AluOpType.mult,
                            accum_out=st_pt[:, k : k + 1],
                        )
                        # balance sum(t) across the two elementwise engines.
                        # (A TensorEngine matmul-with-ones variant simmed 1 us
                        # faster but measured ~10% slower on HW: PE weight-loads
                        # re-read all of t through SBUF ports, contending with
                        # the DMA stream.)
                        if k % 2 == 0:
                            nc.vector.tensor_reduce(
                                out=st_t[:, k : k + 1],
                                in_=t_in[:, :ch],
                                axis=mybir.AxisListType.X,
                                op=mybir.AluOpType.add,
                            )
                        else:
                            tcopy = tmp.tile([P, MAXC], BF16, tag="tcopy")
                            nc.scalar.activation(
                                tcopy[:, :ch],
                                t_in[:, :ch],
                                mybir.ActivationFunctionType.Copy,
                                accum_out=st_t[:, k : k + 1],
                            )
                        off += ch
                        k += 1
            # emitted by _LeanTileContext._drain_and_barrier so the DMA's HBM
            # write receipt overlaps the exit barrier and semaphore clears
            tc.final_dma = (out[:, :], st[:, :])
    nc.compile()
    return nc


def run(pred, target, weight, **spmd_kwargs):
    global _nc_cache
    if _nc_cache is None:
        _nc_cache = _build()
    nc = _nc_cache

    p2 = np.asarray(pred, dtype=np.float32).reshape(B, N)
    t2 = np.asarray(target, dtype=np.float32).reshape(B, N)
    in_maps = []
    for i in range(N_CORES):
        sl = slice(i * SHARD, (i + 1) * SHARD)
        in_maps.append(
            {
                "pred": np.ascontiguousarray(p2[:, sl]).reshape(B, P, F),
                "target": np.ascontiguousarray(t2[:, sl]).reshape(B, P, F),
            }
        )
    res = run_bass_kernel_spmd(nc, in_maps, core_ids=list(range(N_CORES)), **spmd_kwargs)

    partials = np.stack([r["out"] for r in res.results])  # [8, P, 3*NCOLS]
    grp = partials.reshape(N_CORES, P, 3, NCOLS)
    # per-sample sums over cores, partitions, and that sample's chunk columns
    s_b = np.empty((3, B), dtype=np.float64)
    for b in range(B):
        lo, hi = SAMPLE_COL_OFFSETS[b], SAMPLE_COL_OFFSETS[b + 1]
        s_b[:, b] = grp[:, :, :, lo:hi].sum(axis=(0, 1, 3), dtype=np.float64)
    psum, inter, tsum = s_b[0], s_b[1], s_b[2]
    w = np.asarray(weight, dtype=np.float64)
    smooth = 1.0
    dice = (2.0 * inter * w + smooth) / (psum * w + tsum * w + smooth)
    loss = np.sum(1.0 - dice) / B
    return np.array(loss, dtype=np.float32), res


def kernel(pred, target, weight):
    loss, _ = run(pred, target, weight)
    return loss



# revision 2
# speedup vs baseline: 1.0716x; 1.0716x over previous
"""Dice loss (sigmoid + per-sample weighted sums) on 8 Trainium2 NeuronCores.

v2: tapered tail chunks, tensor_scalar sum(t) (DVE 2x mode), SWDGE
prepared/triggered stats writeback, single-ring DMA issue with explicit
transfer order.

Data-parallel: the flattened per-sample element axis (192^3 = 7,077,888) is
sharded contiguously across 8 cores (884,736 elements = [128 x 6912] each).
Each core computes per-partition partial sums of sigmoid(pred), of
sigmoid(pred)*target, and of target for each of the 3 samples; the host sums
the partials and finishes the dice formula (per the data-parallel hint).
"""

import numpy as np

import concourse.bacc as bacc
import concourse.tile as tile
from concourse import mybir
from concourse.bass_utils import run_bass_kernel_spmd
from concourse.tile_sem_assignment import PROC_NAME_TO_IDX
from concourse.vector_clock import ScopedClock, VectorClock

# DMASW lanes in the tile vector clock: the triggered stats-writeback DMA's
# completion tick lives there; the exit drain must not wait on it (the DMA
# only fires at the trigger, after the drain is emitted) — its receipt is
# waited at the very end via the hand-allocated dma_sem instead.
_DMASW_PROCS = [v for k, v in PROC_NAME_TO_IDX.items() if k.startswith("DMASW")]

B = 3                 # batch (samples)
N_CORES = 8
D = 192
N = D * D * D         # 7,077,888 elements per sample
SHARD = N // N_CORES  # 884,736 per core per sample
P = 128               # SBUF partitions
F = SHARD // P        # 6912 free elements per partition

# chunk plan per sample (each list must sum to F); sample 2 tapers so the
# pipeline tail after the last DMA is short
PLANS = [[1728] * 4, [1728] * 4, [1728, 1728, 1728, 576, 432, 432, 288]]
# how many trailing chunks get the "tail" treatment (p prefetched ahead of
# the t streams; see DMA-order construction below)
TAIL = 3
# sum(t) engine/op per chunk index policy: "ts" = DVE tensor_scalar (2x
# mode), "tcopy" = ACT copy+accum, "reduce" = DVE tensor_reduce
SUMT = "ts"
# tail-chunk widths (excluding the last chunk) whose sum(t) moves to ACT
_TCOPY_WIDTHS = {432}
IO_BUFS = 9
TMP_BUFS = 4
# chunk 0's DMAs issued before the TileContext entry barrier would save the
# ~0.7us prologue wait, but the Tile deadlock probe cannot see out-of-context
# semaphore increments and rejects the build; disabled.
PRE_ISSUE = False
FP32 = mybir.dt.float32
BF16 = mybir.dt.bfloat16
INT32 = mybir.dt.int32

NCOLS = sum(len(p) for p in PLANS)          # stat columns per quantity
SAMPLE_COL_OFFSETS = np.cumsum([0] + [len(p) for p in PLANS])
MAXC = max(max(p) for p in PLANS)

_nc_cache = None


class _LeanTileContext(tile.TileContext):
    """Tile exit for single-TileContext kernels:

    1. The stats output leaves via a SWDGE DMA prepared by the idle Pool
       engine near kernel start (deferred-dep prep emitted last) and merely
       *triggered* after the final accumulate, so the tail pays Pool-seq +
       transfer + sem-prop instead of the HWDGE + DGE-delay chain.
    2. The exit drain skips the DMASW completion lanes (the writeback only
       fires at the trigger); its receipt is waited at the very end on gpsimd
       so it overlaps the barrier and semaphore clears.
    3. The trailing all-engine barrier is dropped and PE excluded (single
       TileContext, PE unused).

    NRT re-executes a NEFF only after every engine halted, and gpsimd halts
    after the clears + receipt wait, so re-execution is safe."""

    out_dma_sem = None  # set by _build
    pre_sems = ()       # hand sems of pre-issued chunk-0 DMAs, cleared at exit

    def _drain_and_barrier(self, tick_clock, wait_clock):
        nc = self.nc
        gc = tick_clock.global_clock
        vec = [gc[p] for p in range(len(gc))]
        for p in _DMASW_PROCS:
            vec[p] = 0
        lean_clock = ScopedClock({None: VectorClock(vec)})
        # the stats writers run on ACT/DVE; the prep's desc-gen on Pool.
        # Waits on those three engine lanes imply everything else the
        # trigger needs (every DMA's completion precedes its consumer's
        # engine tick), and fewer wait ops cost less Pool-seq time.
        tvec = [0] * len(vec)
        for name in ("Activation", "DVE", "Pool"):
            p = PROC_NAME_TO_IDX[name]
            tvec[p] = vec[p]
        trig_clock = ScopedClock({None: VectorClock(tvec)})
        dma_sem = self.out_dma_sem
        # Fire the prepared stats writeback, gated on every engine's tile
        # clock (covers all stats writers — and, via the Pool lane, the
        # prep's descriptor generation; the stats buffer itself is a raw
        # SBUF tensor outside Tile tracking).  count=1 is rejected by this
        # runtime; count=None fires all pending preps.
        trig = nc.gpsimd.trigger_dma(count=None)
        wait_clock.add_sem_waits(trig.ins, trig_clock)
        drain_inst = nc.sync.drain()
        wait_clock.add_sem_waits(drain_inst.ins, lean_clock)
        nc.multi_engine_barrier(
            [
                mybir.EngineType.SP,
                mybir.EngineType.Activation,
                mybir.EngineType.DVE,
                mybir.EngineType.Pool,
            ]
        )
        popped = nc._tile_sem_poison_stack.pop()
        assert popped is self._sem_poison
        nc.clear_and_free_semaphores(list(self.sems.allocated().values()))
        nc.gpsimd.wait_ge(dma_sem, 16)
        nc.gpsimd.sem_clear(dma_sem)
        for s in self.pre_sems:
            nc.gpsimd.sem_clear(s)


def _chunk_list():
    """[(b, k, off, ch)] in processing order."""
    out = []
    k = 0
    for b, plan in enumerate(PLANS):
        off = 0
        for ch in plan:
            out.append((b, k, off, ch))
            off += ch
            k += 1
    return out


def _make_nc():
    """Bacc() materializes four [128,1] const APs via gpsimd memsets in the
    prologue, serialized on Pool before the entry barrier (~95ns engine +
    61ns seq each).  This kernel only ever reads const fp32 0.0 (the sigmoid
    bias); skip the other three so the first input DMA issues earlier."""
    import concourse.bass as bassmod

    cls = bassmod.BassEitherVectorEngine
    orig = cls.memset
    skip = ("const-float32-1.0", "const-bfloat16-1.0", "const-uint8-127")

    def patched(self, ap, constant):
        name = getattr(getattr(ap, "tensor", None), "name", "") or ""
        if any(s in name for s in skip):
            return None
        return orig(self, ap, constant)

    cls.memset = patched
    try:
        return bacc.Bacc("TRN2")
    finally:
        cls.memset = orig


def _build(repeat=1):
    nc = _make_nc()
    pred = nc.dram_tensor("pred", [B, P, F], FP32, kind="ExternalInput")
    targ = nc.dram_tensor("target", [B, P, F], FP32, kind="ExternalInput")
    # out[0, :, 0, q*NCOLS + k]: q=0 -> sum sigmoid(p), q=1 -> sum p*t,
    # q=2 -> sum t.  4-D shape matches kv_writeback's
    # [batch, d_head_inner, d_head_outer, n_ctx] output contract.
    out = nc.dram_tensor("out", [1, P, 1, 3 * NCOLS], FP32, kind="ExternalOutput")

    dma_sem = nc.alloc_semaphore("out_dma_sem")

    chunks = _chunk_list()
    n_chunks = len(chunks)

    # Stats + writeback-index buffers live OUTSIDE Tile tracking: the
    # prepare-only writeback below must not acquire data deps on the stats
    # writers (Tile would gate the writers on the DMA's completion — a
    # cycle); ordering is enforced by hand at the trigger instead.
    st_cm = nc.sbuf_tensor("st", [P, 1, 1, 3 * NCOLS], FP32)
    ctx_cm = nc.sbuf_tensor("ctx", [P, 1], INT32)
    st_h = st_cm.__enter__()
    ctx_h = ctx_cm.__enter__()
    cms = [ctx_cm, st_cm]
    pre_dma_names = []
    pre_sems = []
    pre_bufs = {}
    if PRE_ISSUE:
        b0, k0, off0, ch0 = chunks[0]
        p0_cm = nc.sbuf_tensor("p0buf", [P, ch0], FP32)
        t0_cm = nc.sbuf_tensor("t0buf", [P, ch0], FP32)
        p0_h = p0_cm.__enter__()
        t0_h = t0_cm.__enter__()
        cms = [t0_cm, p0_cm] + cms
        p0_sem = nc.alloc_semaphore("p0_sem")
        t0_sem = nc.alloc_semaphore("t0_sem")
        pre_sems = [p0_sem, t0_sem]
        d0 = nc.sync.dma_start(out=p0_h[:, :], in_=pred[b0, :, 0:ch0])
        d0.then_inc(p0_sem, 16)
        d1 = nc.sync.dma_start(out=t0_h[:, :], in_=targ[b0, :, 0:ch0])
        d1.then_inc(t0_sem, 16)
        pre_dma_names = [d0.ins.name, d1.ins.name]
        pre_bufs = {"p": p0_h[:, :], "t": t0_h[:, :]}
    st = st_h[:, :, :, :]
    ctx = ctx_h[:, :]
    st_p = st_h[:, 0, 0, 0:NCOLS]
    st_pt = st_h[:, 0, 0, NCOLS : 2 * NCOLS]
    st_t = st_h[:, 0, 0, 2 * NCOLS : 3 * NCOLS]

    with _LeanTileContext(nc) as tc:
        tc.out_dma_sem = dma_sem
        tc.pre_sems = pre_sems
        with (
            tc.tile_pool(name="io", bufs=IO_BUFS) as io,
            tc.tile_pool(name="tmp", bufs=TMP_BUFS) as tmp,
        ):
            # descriptor prep on the idle Pool engine at kernel start; the
            # exit merely triggers it.  ctx memset precedes the prep in Pool
            # program order (the prep's ucode reads it).
            nc.gpsimd.memset(ctx, 0)
            prep = nc.gpsimd.kv_writeback(
                out[:, :, :, :],
                st,
                ctx,
                prepare_only=True,
                sem=dma_sem,
            )
            prep_name = prep.ins.name

            def _unfence(inst):
                # The prep only reads st's *addresses* at desc-gen time (the
                # data is read when the exit trigger fires, which hand-waits
                # on every engine's tile clock), so the WAR edge the overlap
                # tracker adds from each stats writer to the prep — whose
                # completion signal is the writeback DMA itself — must go, or
                # the first writer deadlocks against its own output DMA.
                # Deps on the pre-issued (out-of-context, tick-less) chunk-0
                # DMAs are likewise stripped; explicit wait_ge's stand in.
                inst.ins.try_remove_dependency(prep_name)
                for n in pre_dma_names:
                    inst.ins.try_remove_dependency(n)
                return inst

            for _ in range(repeat):
                # --- allocate tiles and issue DMAs in explicit transfer
                # order on the single SP HWDGE ring.  Body chunks issue
                # [p_k, t_k] pairs; the last TAIL chunks issue all their p's
                # first (so sigmoids complete early), then their t's, so the
                # final transfers feed only short DVE ops.
                p_tiles, t_tiles = {}, {}
                for b, k, off, ch in chunks:
                    if k == 0 and PRE_ISSUE:
                        p_tiles[k] = pre_bufs["p"]
                        t_tiles[k] = pre_bufs["t"]
                        continue
                    p_tiles[k] = io.tile([P, MAXC], FP32, tag="p_in", name=f"p_in{k}")
                    t_tiles[k] = io.tile([P, MAXC], FP32, tag="t_in", name=f"t_in{k}")

                body = chunks[: n_chunks - TAIL]
                tail = chunks[n_chunks - TAIL :]
                dma_order = []
                for b, k, off, ch in body:
                    dma_order.append(("p", b, k, off, ch))
                    dma_order.append(("t", b, k, off, ch))
                for b, k, off, ch in tail:
                    dma_order.append(("p", b, k, off, ch))
                for b, k, off, ch in tail:
                    dma_order.append(("t", b, k, off, ch))
                for which, b, k, off, ch in dma_order:
                    if k == 0 and PRE_ISSUE:
                        continue  # issued before context entry
                    cols = slice(off, off + ch)
                    if which == "p":
                        nc.sync.dma_start(
                            out=p_tiles[k][:, :ch], in_=pred[b, :, cols]
                        )
                    else:
                        nc.sync.dma_start(
                            out=t_tiles[k][:, :ch], in_=targ[b, :, cols]
                        )

                # --- compute, in chunk order
                for b, k, off, ch in chunks:
                    # tail chunks (except the very last) put sum(t) on ACT:
                    # their t's land in the final ~1.2us of the stream and
                    # DVE alone can't drain stt+sum(t) for all of them; ACT
                    # is idle once the prefetched tail sigmoids are done.
                    sumt = SUMT
                    if n_chunks - TAIL <= k < n_chunks - 1 and ch in _TCOPY_WIDTHS:
                        sumt = "tcopy"
                    p_in, t_in = p_tiles[k], t_tiles[k]
                    if k == 0 and PRE_ISSUE:
                        nc.scalar.wait_ge(pre_sems[0], 16)
                        nc.vector.wait_ge(pre_sems[1], 16)
                    sig = tmp.tile([P, MAXC], FP32, tag="sig")
                    _unfence(nc.scalar.activation(
                        sig[:, :ch],
                        p_in[:, :ch],
                        mybir.ActivationFunctionType.Sigmoid,
                        accum_out=st_p[:, k : k + 1],
                    ))
                    # sum(t) first: it needs only t, so DVE starts it while
                    # the product below still waits on the sigmoid's sem.
                    # prod/tout are discarded side-outputs of the fused
                    # accumulate ops: bf16 halves their SBUF write traffic
                    if sumt == "ts":
                        tout = tmp.tile([P, MAXC], BF16, tag="tout")
                        _unfence(nc.vector.tensor_scalar(
                            out=tout[:, :ch],
                            in0=t_in[:, :ch],
                            scalar1=1.0,
                            scalar2=0.0,
                            op0=mybir.AluOpType.mult,
                            op1=mybir.AluOpType.add,
                            accum_out=st_t[:, k : k + 1],
                        ))
                    elif sumt == "reduce":
                        _unfence(nc.vector.tensor_reduce(
                            out=st_t[:, k : k + 1],
                            in_=t_in[:, :ch],
                            axis=mybir.AxisListType.X,
                            op=mybir.AluOpType.add,
                        ))
                    else:  # tcopy on ACT
                        tout = tmp.tile([P, MAXC], BF16, tag="tout")
                        _unfence(nc.scalar.activation(
                            tout[:, :ch],
                            t_in[:, :ch],
                            mybir.ActivationFunctionType.Copy,
                            accum_out=st_t[:, k : k + 1],
                        ))
                    prod = tmp.tile([P, MAXC], BF16, tag="prod")
                    _unfence(nc.vector.scalar_tensor_tensor(
                        out=prod[:, :ch],
                        in0=sig[:, :ch],
                        scalar=0.0,
                        in1=t_in[:, :ch],
                        op0=mybir.AluOpType.bypass,
                        op1=mybir.AluOpType.mult,
                        accum_out=st_pt[:, k : k + 1],
                    ))

    for cm in cms:
        cm.__exit__(None, None, None)
    nc.compile()
    return nc


def run(pred, target, weight, **spmd_kwargs):
    global _nc_cache
    if _nc_cache is None:
        _nc_cache = _build()
    nc = _nc_cache

    p2 = np.asarray(pred, dtype=np.float32).reshape(B, N)
    t2 = np.asarray(target, dtype=np.float32).reshape(B, N)
    in_maps = []
    for i in range(N_CORES):
        sl = slice(i * SHARD, (i + 1) * SHARD)
        in_maps.append(
            {
                "pred": np.ascontiguousarray(p2[:, sl]).reshape(B, P, F),
                "target": np.ascontiguousarray(t2[:, sl]).reshape(B, P, F),
            }
        )
    res = run_bass_kernel_spmd(nc, in_maps, core_ids=list(range(N_CORES)), **spmd_kwargs)

    partials = np.stack(
        [np.asarray(r["out"]).reshape(P, 3 * NCOLS) for r in res.results]
    )  # [8, P, 3*NCOLS]
    grp = partials.reshape(N_CORES, P, 3, NCOLS)
    # per-sample sums over cores, partitions, and that sample's chunk columns
    s_b = np.empty((3, B), dtype=np.float64)
    for b in range(B):
        lo, hi = SAMPLE_COL_OFFSETS[b], SAMPLE_COL_OFFSETS[b + 1]
        s_b[:, b] = grp[:, :, :, lo:hi].sum(axis=(0, 1, 3), dtype=np.float64)
    psum, inter, tsum = s_b[0], s_b[1], s_b[2]
    w = np.asarray(weight, dtype=np.float64)
    smooth = 1.0
    dice = (2.0 * inter * w + smooth) / (psum * w + tsum * w + smooth)
    loss = np.sum(1.0 - dice) / B
    return np.array(loss, dtype=np.float32), res


def kernel(pred, target, weight):
    loss, _ = run(pred, target, weight)
    return loss


# revision 4
# speedup vs baseline: 1.0790x; 1.0069x over previous
"""Dice loss (sigmoid + per-sample weighted sums) on 8 Trainium2 NeuronCores.

v2: tapered tail chunks, tensor_scalar sum(t) (DVE 2x mode), SWDGE
prepared/triggered stats writeback, single-ring DMA issue with explicit
transfer order.

Data-parallel: the flattened per-sample element axis (192^3 = 7,077,888) is
sharded contiguously across 8 cores (884,736 elements = [128 x 6912] each).
Each core computes per-partition partial sums of sigmoid(pred), of
sigmoid(pred)*target, and of target for each of the 3 samples; the host sums
the partials and finishes the dice formula (per the data-parallel hint).
"""

import numpy as np

import concourse.bacc as bacc
import concourse.tile as tile
from concourse import mybir
from concourse.bass_utils import run_bass_kernel_spmd
from concourse.tile_sem_assignment import PROC_NAME_TO_IDX
from concourse.vector_clock import ScopedClock, VectorClock

# DMASW lanes in the tile vector clock: the triggered stats-writeback DMA's
# completion tick lives there; the exit drain must not wait on it (the DMA
# only fires at the trigger, after the drain is emitted) — its receipt is
# waited at the very end via the hand-allocated dma_sem instead.
_DMASW_PROCS = [v for k, v in PROC_NAME_TO_IDX.items() if k.startswith("DMASW")]

B = 3                 # batch (samples)
N_CORES = 8
D = 192
N = D * D * D         # 7,077,888 elements per sample
SHARD = N // N_CORES  # 884,736 per core per sample
P = 128               # SBUF partitions
F = SHARD // P        # 6912 free elements per partition

# chunk plan per sample (each list must sum to F); sample 2 tapers so the
# pipeline tail after the last DMA is short
PLANS = [[1728] * 4, [1728] * 4, [1728, 1728, 1728, 576, 408, 408, 336]]
# how many trailing chunks get the "tail" treatment (p prefetched ahead of
# the t streams; see DMA-order construction below)
TAIL = 3
# sum(t) engine/op per chunk index policy: "ts" = DVE tensor_scalar (2x
# mode), "tcopy" = ACT copy+accum, "reduce" = DVE tensor_reduce
SUMT = "ts"
# tail-chunk widths (excluding the last chunk) whose sum(t) moves to ACT
_TCOPY_WIDTHS = {432}
# explicit chunk indices whose sum(t) moves to ACT (overrides _TCOPY_WIDTHS
# when non-None): the first mid-tail chunk and the final chunk measured
# fastest on ACT, with the other mid-tail chunk's sum(t) staying on DVE
_TCOPY_IDX = {12, 14}
IO_BUFS = 9
TMP_BUFS = 4
# chunk 0's DMAs issued before the TileContext entry barrier would save the
# ~0.7us prologue wait, but the Tile deadlock probe cannot see out-of-context
# semaphore increments and rejects the build; disabled.
PRE_ISSUE = False
FP32 = mybir.dt.float32
BF16 = mybir.dt.bfloat16
INT32 = mybir.dt.int32

NCOLS = sum(len(p) for p in PLANS)          # stat columns per quantity
SAMPLE_COL_OFFSETS = np.cumsum([0] + [len(p) for p in PLANS])
MAXC = max(max(p) for p in PLANS)

_nc_cache = None


class _LeanTileContext(tile.TileContext):
    """Tile exit for single-TileContext kernels:

    1. The stats output leaves via a SWDGE DMA prepared by the idle Pool
       engine near kernel start (deferred-dep prep emitted last) and merely
       *triggered* after the final accumulate, so the tail pays Pool-seq +
       transfer + sem-prop instead of the HWDGE + DGE-delay chain.
    2. The exit drain skips the DMASW completion lanes (the writeback only
       fires at the trigger); its receipt is waited at the very end on gpsimd
       so it overlaps the barrier and semaphore clears.
    3. The trailing all-engine barrier is dropped and PE excluded (single
       TileContext, PE unused).

    NRT re-executes a NEFF only after every engine halted, and gpsimd halts
    after the clears + receipt wait, so re-execution is safe."""

    out_dma_sem = None  # set by _build
    pre_sems = ()       # hand sems of pre-issued chunk-0 DMAs, cleared at exit

    def _drain_and_barrier(self, tick_clock, wait_clock):
        nc = self.nc
        gc = tick_clock.global_clock
        vec = [gc[p] for p in range(len(gc))]
        for p in _DMASW_PROCS:
            vec[p] = 0
        lean_clock = ScopedClock({None: VectorClock(vec)})
        # the stats writers run on ACT/DVE; the prep's desc-gen on Pool.
        # Waits on those three engine lanes imply everything else the
        # trigger needs (every DMA's completion precedes its consumer's
        # engine tick), and fewer wait ops cost less Pool-seq time.
        tvec = [0] * len(vec)
        for name in ("Activation", "DVE", "Pool"):
            p = PROC_NAME_TO_IDX[name]
            tvec[p] = vec[p]
        trig_clock = ScopedClock({None: VectorClock(tvec)})
        dma_sem = self.out_dma_sem
        # Fire the prepared stats writeback, gated on every engine's tile
        # clock (covers all stats writers — and, via the Pool lane, the
        # prep's descriptor generation; the stats buffer itself is a raw
        # SBUF tensor outside Tile tracking).  count=1 is rejected by this
        # runtime; count=None fires all pending preps.
        trig = nc.gpsimd.trigger_dma(count=None)
        wait_clock.add_sem_waits(trig.ins, trig_clock)
        drain_inst = nc.sync.drain()
        wait_clock.add_sem_waits(drain_inst.ins, lean_clock)
        nc.multi_engine_barrier(
            [
                mybir.EngineType.SP,
                mybir.EngineType.Activation,
                mybir.EngineType.DVE,
                mybir.EngineType.Pool,
            ]
        )
        popped = nc._tile_sem_poison_stack.pop()
        assert popped is self._sem_poison
        nc.clear_and_free_semaphores(list(self.sems.allocated().values()))
        nc.gpsimd.wait_ge(dma_sem, 16)
        nc.gpsimd.sem_clear(dma_sem)
        for s in self.pre_sems:
            nc.gpsimd.sem_clear(s)


def _chunk_list():
    """[(b, k, off, ch)] in processing order."""
    out = []
    k = 0
    for b, plan in enumerate(PLANS):
        off = 0
        for ch in plan:
            out.append((b, k, off, ch))
            off += ch
            k += 1
    return out


def _make_nc():
    """Bacc() materializes four [128,1] const APs via gpsimd memsets in the
    prologue, serialized on Pool before the prologue barrier (~95ns engine +
    61ns seq each).  This kernel only ever reads const fp32 0.0 (the sigmoid
    bias); skip the other three so the first input DMA issues earlier.

    The prologue all-engine barrier also excludes SP: it fences the const
    memsets and entry semaphore clears against use, but SP's first
    instructions are input DMAs that wait on nothing, increment semaphores
    only microseconds later (HWDGE + DGE latency + transfer), and every
    semaphore they touch was already cleared by the previous execution's
    exit sequence — so SP can start issuing immediately."""
    import concourse.bass as bassmod

    cls = bassmod.BassEitherVectorEngine
    orig_memset = cls.memset
    skip = ("const-float32-1.0", "const-bfloat16-1.0", "const-uint8-127")

    def patched_memset(self, ap, constant):
        name = getattr(getattr(ap, "tensor", None), "name", "") or ""
        if any(s in name for s in skip):
            return None
        return orig_memset(self, ap, constant)

    orig_aeb = bacc.Bacc.all_engine_barrier

    def patched_aeb(self, *, sem_only: bool = False):
        self.multi_engine_barrier(
            [e for e in self.engines if e != mybir.EngineType.SP]
        )

    cls.memset = patched_memset
    bacc.Bacc.all_engine_barrier = patched_aeb
    try:
        return bacc.Bacc("TRN2")
    finally:
        cls.memset = orig_memset
        bacc.Bacc.all_engine_barrier = orig_aeb


def _build(repeat=1):
    nc = _make_nc()
    pred = nc.dram_tensor("pred", [B, P, F], FP32, kind="ExternalInput")
    targ = nc.dram_tensor("target", [B, P, F], FP32, kind="ExternalInput")
    # out[0, :, 0, q*NCOLS + k]: q=0 -> sum sigmoid(p), q=1 -> sum p*t,
    # q=2 -> sum t.  4-D shape matches kv_writeback's
    # [batch, d_head_inner, d_head_outer, n_ctx] output contract.
    out = nc.dram_tensor("out", [1, P, 1, 3 * NCOLS], FP32, kind="ExternalOutput")

    dma_sem = nc.alloc_semaphore("out_dma_sem")

    chunks = _chunk_list()
    n_chunks = len(chunks)

    # Stats + writeback-index buffers live OUTSIDE Tile tracking: the
    # prepare-only writeback below must not acquire data deps on the stats
    # writers (Tile would gate the writers on the DMA's completion — a
    # cycle); ordering is enforced by hand at the trigger instead.
    st_cm = nc.sbuf_tensor("st", [P, 1, 1, 3 * NCOLS], FP32)
    ctx_cm = nc.sbuf_tensor("ctx", [P, 1], INT32)
    st_h = st_cm.__enter__()
    ctx_h = ctx_cm.__enter__()
    cms = [ctx_cm, st_cm]
    pre_dma_names = []
    pre_sems = []
    pre_bufs = {}
    if PRE_ISSUE:
        b0, k0, off0, ch0 = chunks[0]
        p0_cm = nc.sbuf_tensor("p0buf", [P, ch0], FP32)
        t0_cm = nc.sbuf_tensor("t0buf", [P, ch0], FP32)
        p0_h = p0_cm.__enter__()
        t0_h = t0_cm.__enter__()
        cms = [t0_cm, p0_cm] + cms
        p0_sem = nc.alloc_semaphore("p0_sem")
        t0_sem = nc.alloc_semaphore("t0_sem")
        pre_sems = [p0_sem, t0_sem]
        d0 = nc.sync.dma_start(out=p0_h[:, :], in_=pred[b0, :, 0:ch0])
        d0.then_inc(p0_sem, 16)
        d1 = nc.sync.dma_start(out=t0_h[:, :], in_=targ[b0, :, 0:ch0])
        d1.then_inc(t0_sem, 16)
        pre_dma_names = [d0.ins.name, d1.ins.name]
        pre_bufs = {"p": p0_h[:, :], "t": t0_h[:, :]}
    st = st_h[:, :, :, :]
    ctx = ctx_h[:, :]
    st_p = st_h[:, 0, 0, 0:NCOLS]
    st_pt = st_h[:, 0, 0, NCOLS : 2 * NCOLS]
    st_t = st_h[:, 0, 0, 2 * NCOLS : 3 * NCOLS]

    with _LeanTileContext(nc) as tc:
        tc.out_dma_sem = dma_sem
        tc.pre_sems = pre_sems
        with (
            tc.tile_pool(name="io", bufs=IO_BUFS) as io,
            tc.tile_pool(name="tmp", bufs=TMP_BUFS) as tmp,
        ):
            # descriptor prep on the idle Pool engine at kernel start; the
            # exit merely triggers it.  ctx memset precedes the prep in Pool
            # program order (the prep's ucode reads it).
            nc.gpsimd.memset(ctx, 0)
            prep = nc.gpsimd.kv_writeback(
                out[:, :, :, :],
                st,
                ctx,
                prepare_only=True,
                sem=dma_sem,
            )
            prep_name = prep.ins.name

            def _unfence(inst):
                # The prep only reads st's *addresses* at desc-gen time (the
                # data is read when the exit trigger fires, which hand-waits
                # on every engine's tile clock), so the WAR edge the overlap
                # tracker adds from each stats writer to the prep — whose
                # completion signal is the writeback DMA itself — must go, or
                # the first writer deadlocks against its own output DMA.
                # Deps on the pre-issued (out-of-context, tick-less) chunk-0
                # DMAs are likewise stripped; explicit wait_ge's stand in.
                inst.ins.try_remove_dependency(prep_name)
                for n in pre_dma_names:
                    inst.ins.try_remove_dependency(n)
                return inst

            for _ in range(repeat):
                # --- allocate tiles and issue DMAs in explicit transfer
                # order on the single SP HWDGE ring.  Body chunks issue
                # [p_k, t_k] pairs; the last TAIL chunks issue all their p's
                # first (so sigmoids complete early), then their t's, so the
                # final transfers feed only short DVE ops.
                p_tiles, t_tiles = {}, {}
                for b, k, off, ch in chunks:
                    if k == 0 and PRE_ISSUE:
                        p_tiles[k] = pre_bufs["p"]
                        t_tiles[k] = pre_bufs["t"]
                        continue
                    p_tiles[k] = io.tile([P, MAXC], FP32, tag="p_in", name=f"p_in{k}")
                    t_tiles[k] = io.tile([P, MAXC], FP32, tag="t_in", name=f"t_in{k}")

                body = chunks[: n_chunks - TAIL]
                tail = chunks[n_chunks - TAIL :]
                dma_order = []
                for b, k, off, ch in body:
                    dma_order.append(("p", b, k, off, ch))
                    dma_order.append(("t", b, k, off, ch))
                for b, k, off, ch in tail:
                    dma_order.append(("p", b, k, off, ch))
                for b, k, off, ch in tail:
                    dma_order.append(("t", b, k, off, ch))
                for which, b, k, off, ch in dma_order:
                    if k == 0 and PRE_ISSUE:
                        continue  # issued before context entry
                    cols = slice(off, off + ch)
                    if which == "p":
                        nc.sync.dma_start(
                            out=p_tiles[k][:, :ch], in_=pred[b, :, cols]
                        )
                    else:
                        nc.sync.dma_start(
                            out=t_tiles[k][:, :ch], in_=targ[b, :, cols]
                        )

                # --- compute, in chunk order
                for b, k, off, ch in chunks:
                    # tail chunks (except the very last) put sum(t) on ACT:
                    # their t's land in the final ~1.2us of the stream and
                    # DVE alone can't drain stt+sum(t) for all of them; ACT
                    # is idle once the prefetched tail sigmoids are done.
                    sumt = SUMT
                    if _TCOPY_IDX is not None:
                        if k in _TCOPY_IDX:
                            sumt = "tcopy"
                    elif n_chunks - TAIL <= k < n_chunks - 1 and ch in _TCOPY_WIDTHS:
                        sumt = "tcopy"
                    p_in, t_in = p_tiles[k], t_tiles[k]
                    if k == 0 and PRE_ISSUE:
                        nc.scalar.wait_ge(pre_sems[0], 16)
                        nc.vector.wait_ge(pre_sems[1], 16)
                    sig = tmp.tile([P, MAXC], FP32, tag="sig")
                    _unfence(nc.scalar.activation(
                        sig[:, :ch],
                        p_in[:, :ch],
                        mybir.ActivationFunctionType.Sigmoid,
                        accum_out=st_p[:, k : k + 1],
                    ))
                    # sum(t) first: it needs only t, so DVE starts it while
                    # the product below still waits on the sigmoid's sem.
                    # prod/tout are discarded side-outputs of the fused
                    # accumulate ops: bf16 halves their SBUF write traffic
                    if sumt == "ts":
                        tout = tmp.tile([P, MAXC], BF16, tag="tout")
                        _unfence(nc.vector.tensor_scalar(
                            out=tout[:, :ch],
                            in0=t_in[:, :ch],
                            scalar1=1.0,
                            scalar2=0.0,
                            op0=mybir.AluOpType.mult,
                            op1=mybir.AluOpType.add,
                            accum_out=st_t[:, k : k + 1],
                        ))
                    elif sumt == "reduce":
                        _unfence(nc.vector.tensor_reduce(
                            out=st_t[:, k : k + 1],
                            in_=t_in[:, :ch],
                            axis=mybir.AxisListType.X,
                            op=mybir.AluOpType.add,
                        ))
                    else:  # tcopy on ACT
                        tout = tmp.tile([P, MAXC], BF16, tag="tout")
                        _unfence(nc.scalar.activation(
                            tout[:, :ch],
                            t_in[:, :ch],
                            mybir.ActivationFunctionType.Copy,
                            accum_out=st_t[:, k : k + 1],
                        ))
                    prod = tmp.tile([P, MAXC], BF16, tag="prod")
                    _unfence(nc.vector.scalar_tensor_tensor(
                        out=prod[:, :ch],
                        in0=sig[:, :ch],
                        scalar=0.0,
                        in1=t_in[:, :ch],
                        op0=mybir.AluOpType.bypass,
                        op1=mybir.AluOpType.mult,
                        accum_out=st_pt[:, k : k + 1],
                    ))

    for cm in cms:
        cm.__exit__(None, None, None)
    nc.compile()
    return nc


def run(pred, target, weight, **spmd_kwargs):
    global _nc_cache
    if _nc_cache is None:
        _nc_cache = _build()
    nc = _nc_cache

    p2 = np.asarray(pred, dtype=np.float32).reshape(B, N)
    t2 = np.asarray(target, dtype=np.float32).reshape(B, N)
    in_maps = []
    for i in range(N_CORES):
        sl = slice(i * SHARD, (i + 1) * SHARD)
        in_maps.append(
            {
                "pred": np.ascontiguousarray(p2[:, sl]).reshape(B, P, F),
                "target": np.ascontiguousarray(t2[:, sl]).reshape(B, P, F),
            }
        )
    res = run_bass_kernel_spmd(nc, in_maps, core_ids=list(range(N_CORES)), **spmd_kwargs)

    partials = np.stack(
        [np.asarray(r["out"]).reshape(P, 3 * NCOLS) for r in res.results]
    )  # [8, P, 3*NCOLS]
    grp = partials.reshape(N_CORES, P, 3, NCOLS)
    # per-sample sums over cores, partitions, and that sample's chunk columns
    s_b = np.empty((3, B), dtype=np.float64)
    for b in range(B):
        lo, hi = SAMPLE_COL_OFFSETS[b], SAMPLE_COL_OFFSETS[b + 1]
        s_b[:, b] = grp[:, :, :, lo:hi].sum(axis=(0, 1, 3), dtype=np.float64)
    psum, inter, tsum = s_b[0], s_b[1], s_b[2]
    w = np.asarray(weight, dtype=np.float64)
    smooth = 1.0
    dice = (2.0 * inter * w + smooth) / (psum * w + tsum * w + smooth)
    loss = np.sum(1.0 - dice) / B
    return np.array(loss, dtype=np.float32), res


def kernel(pred, target, weight):
    loss, _ = run(pred, target, weight)
    return loss


# revision 5
# speedup vs baseline: 1.0797x; 1.0007x over previous
"""Dice loss (sigmoid + per-sample weighted sums) on 8 Trainium2 NeuronCores.

v2: tapered tail chunks, tensor_scalar sum(t) (DVE 2x mode), SWDGE
prepared/triggered stats writeback, single-ring DMA issue with explicit
transfer order.

Data-parallel: the flattened per-sample element axis (192^3 = 7,077,888) is
sharded contiguously across 8 cores (884,736 elements = [128 x 6912] each).
Each core computes per-partition partial sums of sigmoid(pred), of
sigmoid(pred)*target, and of target for each of the 3 samples; the host sums
the partials and finishes the dice formula (per the data-parallel hint).
"""

import numpy as np

import concourse.bacc as bacc
import concourse.tile as tile
from concourse import mybir
from concourse.bass_utils import run_bass_kernel_spmd
from concourse.tile_sem_assignment import PROC_NAME_TO_IDX
from concourse.vector_clock import ScopedClock, VectorClock

# DMASW lanes in the tile vector clock: the triggered stats-writeback DMA's
# completion tick lives there; the exit drain must not wait on it (the DMA
# only fires at the trigger, after the drain is emitted) — its receipt is
# waited at the very end via the hand-allocated dma_sem instead.
_DMASW_PROCS = [v for k, v in PROC_NAME_TO_IDX.items() if k.startswith("DMASW")]

B = 3                 # batch (samples)
N_CORES = 8
D = 192
N = D * D * D         # 7,077,888 elements per sample
SHARD = N // N_CORES  # 884,736 per core per sample
P = 128               # SBUF partitions
F = SHARD // P        # 6912 free elements per partition

# chunk plan per sample (each list must sum to F); sample 2 tapers so the
# pipeline tail after the last DMA is short
PLANS = [[1728] * 4, [1728] * 4, [1728, 1728, 1728, 576, 384, 384, 384]]
# how many trailing chunks get the "tail" treatment (p prefetched ahead of
# the t streams; see DMA-order construction below)
TAIL = 3
# sum(t) engine/op per chunk index policy: "ts" = DVE tensor_scalar (2x
# mode), "tcopy" = ACT copy+accum, "reduce" = DVE tensor_reduce
SUMT = "ts"
# tail-chunk widths (excluding the last chunk) whose sum(t) moves to ACT
_TCOPY_WIDTHS = {432}
# explicit chunk indices whose sum(t) moves to ACT (overrides _TCOPY_WIDTHS
# when non-None): the first mid-tail chunk and the final chunk measured
# fastest on ACT, with the other mid-tail chunk's sum(t) staying on DVE
_TCOPY_IDX = {12, 14}
IO_BUFS = 9
TMP_BUFS = 4
# chunk 0's DMAs issued before the TileContext entry barrier would save the
# ~0.7us prologue wait, but the Tile deadlock probe cannot see out-of-context
# semaphore increments and rejects the build; disabled.
PRE_ISSUE = False
FP32 = mybir.dt.float32
BF16 = mybir.dt.bfloat16
INT32 = mybir.dt.int32

NCOLS = sum(len(p) for p in PLANS)          # stat columns per quantity
SAMPLE_COL_OFFSETS = np.cumsum([0] + [len(p) for p in PLANS])
MAXC = max(max(p) for p in PLANS)

_nc_cache = None


class _LeanTileContext(tile.TileContext):
    """Tile exit for single-TileContext kernels:

    1. The stats output leaves via a SWDGE DMA prepared by the idle Pool
       engine near kernel start (deferred-dep prep emitted last) and merely
       *triggered* after the final accumulate, so the tail pays Pool-seq +
       transfer + sem-prop instead of the HWDGE + DGE-delay chain.
    2. The exit drain skips the DMASW completion lanes (the writeback only
       fires at the trigger); its receipt is waited at the very end on gpsimd
       so it overlaps the barrier and semaphore clears.
    3. The trailing all-engine barrier is dropped and PE excluded (single
       TileContext, PE unused).

    NRT re-executes a NEFF only after every engine halted, and gpsimd halts
    after the clears + receipt wait, so re-execution is safe."""

    out_dma_sem = None  # set by _build
    pre_sems = ()       # hand sems of pre-issued chunk-0 DMAs, cleared at exit

    def _drain_and_barrier(self, tick_clock, wait_clock):
        nc = self.nc
        gc = tick_clock.global_clock
        vec = [gc[p] for p in range(len(gc))]
        for p in _DMASW_PROCS:
            vec[p] = 0
        lean_clock = ScopedClock({None: VectorClock(vec)})
        # the stats writers run on ACT/DVE; the prep's desc-gen on Pool.
        # Waits on those three engine lanes imply everything else the
        # trigger needs (every DMA's completion precedes its consumer's
        # engine tick), and fewer wait ops cost less Pool-seq time.
        tvec = [0] * len(vec)
        for name in ("Activation", "DVE", "Pool"):
            p = PROC_NAME_TO_IDX[name]
            tvec[p] = vec[p]
        trig_clock = ScopedClock({None: VectorClock(tvec)})
        dma_sem = self.out_dma_sem
        # Fire the prepared stats writeback, gated on every engine's tile
        # clock (covers all stats writers — and, via the Pool lane, the
        # prep's descriptor generation; the stats buffer itself is a raw
        # SBUF tensor outside Tile tracking).  count=1 is rejected by this
        # runtime; count=None fires all pending preps.
        trig = nc.gpsimd.trigger_dma(count=None)
        wait_clock.add_sem_waits(trig.ins, trig_clock)
        drain_inst = nc.sync.drain()
        wait_clock.add_sem_waits(drain_inst.ins, lean_clock)
        nc.multi_engine_barrier(
            [
                mybir.EngineType.SP,
                mybir.EngineType.Activation,
                mybir.EngineType.DVE,
                mybir.EngineType.Pool,
            ]
        )
        popped = nc._tile_sem_poison_stack.pop()
        assert popped is self._sem_poison
        nc.clear_and_free_semaphores(list(self.sems.allocated().values()))
        nc.gpsimd.wait_ge(dma_sem, 16)
        nc.gpsimd.sem_clear(dma_sem)
        for s in self.pre_sems:
            nc.gpsimd.sem_clear(s)


def _chunk_list():
    """[(b, k, off, ch)] in processing order."""
    out = []
    k = 0
    for b, plan in enumerate(PLANS):
        off = 0
        for ch in plan:
            out.append((b, k, off, ch))
            off += ch
            k += 1
    return out


def _make_nc():
    """Bacc() materializes four [128,1] const APs via gpsimd memsets in the
    prologue, serialized on Pool before the prologue barrier (~95ns engine +
    61ns seq each).  This kernel only ever reads const fp32 0.0 (the sigmoid
    bias); skip the other three so the first input DMA issues earlier.

    The prologue all-engine barrier also excludes SP: it fences the const
    memsets and entry semaphore clears against use, but SP's first
    instructions are input DMAs that wait on nothing, increment semaphores
    only microseconds later (HWDGE + DGE latency + transfer), and every
    semaphore they touch was already cleared by the previous execution's
    exit sequence — so SP can start issuing immediately."""
    import concourse.bass as bassmod

    cls = bassmod.BassEitherVectorEngine
    orig_memset = cls.memset
    skip = ("const-float32-1.0", "const-bfloat16-1.0", "const-uint8-127")

    def patched_memset(self, ap, constant):
        name = getattr(getattr(ap, "tensor", None), "name", "") or ""
        if any(s in name for s in skip):
            return None
        return orig_memset(self, ap, constant)

    orig_aeb = bacc.Bacc.all_engine_barrier

    def patched_aeb(self, *, sem_only: bool = False):
        self.multi_engine_barrier(
            [e for e in self.engines if e != mybir.EngineType.SP]
        )

    cls.memset = patched_memset
    bacc.Bacc.all_engine_barrier = patched_aeb
    try:
        return bacc.Bacc("TRN2")
    finally:
        cls.memset = orig_memset
        bacc.Bacc.all_engine_barrier = orig_aeb


def _build(repeat=1):
    nc = _make_nc()
    pred = nc.dram_tensor("pred", [B, P, F], FP32, kind="ExternalInput")
    targ = nc.dram_tensor("target", [B, P, F], FP32, kind="ExternalInput")
    # out[0, :, 0, q*NCOLS + k]: q=0 -> sum sigmoid(p), q=1 -> sum p*t,
    # q=2 -> sum t.  4-D shape matches kv_writeback's
    # [batch, d_head_inner, d_head_outer, n_ctx] output contract.
    out = nc.dram_tensor("out", [1, P, 1, 3 * NCOLS], FP32, kind="ExternalOutput")

    dma_sem = nc.alloc_semaphore("out_dma_sem")

    chunks = _chunk_list()
    n_chunks = len(chunks)

    # Stats + writeback-index buffers live OUTSIDE Tile tracking: the
    # prepare-only writeback below must not acquire data deps on the stats
    # writers (Tile would gate the writers on the DMA's completion — a
    # cycle); ordering is enforced by hand at the trigger instead.
    st_cm = nc.sbuf_tensor("st", [P, 1, 1, 3 * NCOLS], FP32)
    ctx_cm = nc.sbuf_tensor("ctx", [P, 1], INT32)
    st_h = st_cm.__enter__()
    ctx_h = ctx_cm.__enter__()
    cms = [ctx_cm, st_cm]
    pre_dma_names = []
    pre_sems = []
    pre_bufs = {}
    if PRE_ISSUE:
        b0, k0, off0, ch0 = chunks[0]
        p0_cm = nc.sbuf_tensor("p0buf", [P, ch0], FP32)
        t0_cm = nc.sbuf_tensor("t0buf", [P, ch0], FP32)
        p0_h = p0_cm.__enter__()
        t0_h = t0_cm.__enter__()
        cms = [t0_cm, p0_cm] + cms
        p0_sem = nc.alloc_semaphore("p0_sem")
        t0_sem = nc.alloc_semaphore("t0_sem")
        pre_sems = [p0_sem, t0_sem]
        d0 = nc.sync.dma_start(out=p0_h[:, :], in_=pred[b0, :, 0:ch0])
        d0.then_inc(p0_sem, 16)
        d1 = nc.sync.dma_start(out=t0_h[:, :], in_=targ[b0, :, 0:ch0])
        d1.then_inc(t0_sem, 16)
        pre_dma_names = [d0.ins.name, d1.ins.name]
        pre_bufs = {"p": p0_h[:, :], "t": t0_h[:, :]}
    st = st_h[:, :, :, :]
    ctx = ctx_h[:, :]
    st_p = st_h[:, 0, 0, 0:NCOLS]
    st_pt = st_h[:, 0, 0, NCOLS : 2 * NCOLS]
    st_t = st_h[:, 0, 0, 2 * NCOLS : 3 * NCOLS]

    with _LeanTileContext(nc) as tc:
        tc.out_dma_sem = dma_sem
        tc.pre_sems = pre_sems
        with (
            tc.tile_pool(name="io", bufs=IO_BUFS) as io,
            tc.tile_pool(name="tmp", bufs=TMP_BUFS) as tmp,
        ):
            # descriptor prep on the idle Pool engine at kernel start; the
            # exit merely triggers it.  ctx memset precedes the prep in Pool
            # program order (the prep's ucode reads it).
            nc.gpsimd.memset(ctx, 0)
            prep = nc.gpsimd.kv_writeback(
                out[:, :, :, :],
                st,
                ctx,
                prepare_only=True,
                sem=dma_sem,
            )
            prep_name = prep.ins.name

            def _unfence(inst):
                # The prep only reads st's *addresses* at desc-gen time (the
                # data is read when the exit trigger fires, which hand-waits
                # on every engine's tile clock), so the WAR edge the overlap
                # tracker adds from each stats writer to the prep — whose
                # completion signal is the writeback DMA itself — must go, or
                # the first writer deadlocks against its own output DMA.
                # Deps on the pre-issued (out-of-context, tick-less) chunk-0
                # DMAs are likewise stripped; explicit wait_ge's stand in.
                inst.ins.try_remove_dependency(prep_name)
                for n in pre_dma_names:
                    inst.ins.try_remove_dependency(n)
                return inst

            for _ in range(repeat):
                # --- allocate tiles and issue DMAs in explicit transfer
                # order on the single SP HWDGE ring.  Body chunks issue
                # [p_k, t_k] pairs; the last TAIL chunks issue all their p's
                # first (so sigmoids complete early), then their t's, so the
                # final transfers feed only short DVE ops.
                p_tiles, t_tiles = {}, {}
                for b, k, off, ch in chunks:
                    if k == 0 and PRE_ISSUE:
                        p_tiles[k] = pre_bufs["p"]
                        t_tiles[k] = pre_bufs["t"]
                        continue
                    p_tiles[k] = io.tile([P, MAXC], FP32, tag="p_in", name=f"p_in{k}")
                    t_tiles[k] = io.tile([P, MAXC], FP32, tag="t_in", name=f"t_in{k}")

                body = chunks[: n_chunks - TAIL]
                tail = chunks[n_chunks - TAIL :]
                dma_order = []
                for b, k, off, ch in body:
                    dma_order.append(("p", b, k, off, ch))
                    dma_order.append(("t", b, k, off, ch))
                for b, k, off, ch in tail:
                    dma_order.append(("p", b, k, off, ch))
                for b, k, off, ch in tail:
                    dma_order.append(("t", b, k, off, ch))
                for which, b, k, off, ch in dma_order:
                    if k == 0 and PRE_ISSUE:
                        continue  # issued before context entry
                    cols = slice(off, off + ch)
                    if which == "p":
                        nc.sync.dma_start(
                            out=p_tiles[k][:, :ch], in_=pred[b, :, cols]
                        )
                    else:
                        nc.sync.dma_start(
                            out=t_tiles[k][:, :ch], in_=targ[b, :, cols]
                        )

                # --- compute, in chunk order
                for b, k, off, ch in chunks:
                    # tail chunks (except the very last) put sum(t) on ACT:
                    # their t's land in the final ~1.2us of the stream and
                    # DVE alone can't drain stt+sum(t) for all of them; ACT
                    # is idle once the prefetched tail sigmoids are done.
                    sumt = SUMT
                    if _TCOPY_IDX is not None:
                        if k in _TCOPY_IDX:
                            sumt = "tcopy"
                    elif n_chunks - TAIL <= k < n_chunks - 1 and ch in _TCOPY_WIDTHS:
                        sumt = "tcopy"
                    p_in, t_in = p_tiles[k], t_tiles[k]
                    if k == 0 and PRE_ISSUE:
                        nc.scalar.wait_ge(pre_sems[0], 16)
                        nc.vector.wait_ge(pre_sems[1], 16)
                    sig = tmp.tile([P, MAXC], FP32, tag="sig")
                    _unfence(nc.scalar.activation(
                        sig[:, :ch],
                        p_in[:, :ch],
                        mybir.ActivationFunctionType.Sigmoid,
                        accum_out=st_p[:, k : k + 1],
                    ))
                    # sum(t) first: it needs only t, so DVE starts it while
                    # the product below still waits on the sigmoid's sem.
                    # prod/tout are discarded side-outputs of the fused
                    # accumulate ops: bf16 halves their SBUF write traffic
                    if sumt == "ts":
                        tout = tmp.tile([P, MAXC], BF16, tag="tout")
                        _unfence(nc.vector.tensor_scalar(
                            out=tout[:, :ch],
                            in0=t_in[:, :ch],
                            scalar1=1.0,
                            scalar2=0.0,
                            op0=mybir.AluOpType.mult,
                            op1=mybir.AluOpType.add,
                            accum_out=st_t[:, k : k + 1],
                        ))
                    elif sumt == "reduce":
                        _unfence(nc.vector.tensor_reduce(
                            out=st_t[:, k : k + 1],
                            in_=t_in[:, :ch],
                            axis=mybir.AxisListType.X,
                            op=mybir.AluOpType.add,
                        ))
                    else:  # tcopy on ACT
                        tout = tmp.tile([P, MAXC], BF16, tag="tout")
                        _unfence(nc.scalar.activation(
                            tout[:, :ch],
                            t_in[:, :ch],
                            mybir.ActivationFunctionType.Copy,
                            accum_out=st_t[:, k : k + 1],
                        ))
                    prod = tmp.tile([P, MAXC], BF16, tag="prod")
                    _unfence(nc.vector.scalar_tensor_tensor(
                        out=prod[:, :ch],
                        in0=sig[:, :ch],
                        scalar=0.0,
                        in1=t_in[:, :ch],
                        op0=mybir.AluOpType.bypass,
                        op1=mybir.AluOpType.mult,
                        accum_out=st_pt[:, k : k + 1],
                    ))

    for cm in cms:
        cm.__exit__(None, None, None)
    nc.compile()
    return nc


def run(pred, target, weight, **spmd_kwargs):
    global _nc_cache
    if _nc_cache is None:
        _nc_cache = _build()
    nc = _nc_cache

    p2 = np.asarray(pred, dtype=np.float32).reshape(B, N)
    t2 = np.asarray(target, dtype=np.float32).reshape(B, N)
    in_maps = []
    for i in range(N_CORES):
        sl = slice(i * SHARD, (i + 1) * SHARD)
        in_maps.append(
            {
                "pred": np.ascontiguousarray(p2[:, sl]).reshape(B, P, F),
                "target": np.ascontiguousarray(t2[:, sl]).reshape(B, P, F),
            }
        )
    res = run_bass_kernel_spmd(nc, in_maps, core_ids=list(range(N_CORES)), **spmd_kwargs)

    partials = np.stack(
        [np.asarray(r["out"]).reshape(P, 3 * NCOLS) for r in res.results]
    )  # [8, P, 3*NCOLS]
    grp = partials.reshape(N_CORES, P, 3, NCOLS)
    # per-sample sums over cores, partitions, and that sample's chunk columns
    s_b = np.empty((3, B), dtype=np.float64)
    for b in range(B):
        lo, hi = SAMPLE_COL_OFFSETS[b], SAMPLE_COL_OFFSETS[b + 1]
        s_b[:, b] = grp[:, :, :, lo:hi].sum(axis=(0, 1, 3), dtype=np.float64)
    psum, inter, tsum = s_b[0], s_b[1], s_b[2]
    w = np.asarray(weight, dtype=np.float64)
    smooth = 1.0
    dice = (2.0 * inter * w + smooth) / (psum * w + tsum * w + smooth)
    loss = np.sum(1.0 - dice) / B
    return np.array(loss, dtype=np.float32), res


def kernel(pred, target, weight):
    loss, _ = run(pred, target, weight)
    return loss


# revision 6
# speedup vs baseline: 1.0799x; 1.0002x over previous
"""Dice loss (sigmoid + per-sample weighted sums) on 8 Trainium2 NeuronCores.

v2: tapered tail chunks, tensor_scalar sum(t) (DVE 2x mode), SWDGE
prepared/triggered stats writeback, single-ring DMA issue with explicit
transfer order.

Data-parallel: the flattened per-sample element axis (192^3 = 7,077,888) is
sharded contiguously across 8 cores (884,736 elements = [128 x 6912] each).
Each core computes per-partition partial sums of sigmoid(pred), of
sigmoid(pred)*target, and of target for each of the 3 samples; the host sums
the partials and finishes the dice formula (per the data-parallel hint).
"""

import numpy as np

import concourse.bacc as bacc
import concourse.tile as tile
from concourse import mybir
from concourse.bass_utils import run_bass_kernel_spmd
from concourse.tile_sem_assignment import PROC_NAME_TO_IDX
from concourse.vector_clock import ScopedClock, VectorClock

# DMASW lanes in the tile vector clock: the triggered stats-writeback DMA's
# completion tick lives there; the exit drain must not wait on it (the DMA
# only fires at the trigger, after the drain is emitted) — its receipt is
# waited at the very end via the hand-allocated dma_sem instead.
_DMASW_PROCS = [v for k, v in PROC_NAME_TO_IDX.items() if k.startswith("DMASW")]

B = 3                 # batch (samples)
N_CORES = 8
D = 192
N = D * D * D         # 7,077,888 elements per sample
SHARD = N // N_CORES  # 884,736 per core per sample
P = 128               # SBUF partitions
F = SHARD // P        # 6912 free elements per partition

# chunk plan per sample (each list must sum to F); sample 2 tapers so the
# pipeline tail after the last DMA is short
PLANS = [
    [1296, 1296, 1296, 1296, 1296, 432],
    [1296, 1296, 1296, 1296, 1296, 432],
    [1296, 1296, 1296, 1296, 576, 384, 384, 384],
]
# how many trailing chunks get the "tail" treatment (p prefetched ahead of
# the t streams; see DMA-order construction below)
TAIL = 3
# sum(t) engine/op per chunk index policy: "ts" = DVE tensor_scalar (2x
# mode), "tcopy" = ACT copy+accum, "reduce" = DVE tensor_reduce
SUMT = "ts"
# tail-chunk widths (excluding the last chunk) whose sum(t) moves to ACT
_TCOPY_WIDTHS = {432}
# explicit chunk indices whose sum(t) moves to ACT (overrides _TCOPY_WIDTHS
# when non-None): the first mid-tail chunk and the final chunk measured
# fastest on ACT, with the other mid-tail chunk's sum(t) staying on DVE
_TCOPY_IDX = {17, 19}
IO_BUFS = 9
TMP_BUFS = 4
# chunk 0's DMAs issued before the TileContext entry barrier would save the
# ~0.7us prologue wait, but the Tile deadlock probe cannot see out-of-context
# semaphore increments and rejects the build; disabled.
PRE_ISSUE = False
FP32 = mybir.dt.float32
BF16 = mybir.dt.bfloat16
INT32 = mybir.dt.int32

NCOLS = sum(len(p) for p in PLANS)          # stat columns per quantity
SAMPLE_COL_OFFSETS = np.cumsum([0] + [len(p) for p in PLANS])
MAXC = max(max(p) for p in PLANS)

_nc_cache = None


class _LeanTileContext(tile.TileContext):
    """Tile exit for single-TileContext kernels:

    1. The stats output leaves via a SWDGE DMA prepared by the idle Pool
       engine near kernel start (deferred-dep prep emitted last) and merely
       *triggered* after the final accumulate, so the tail pays Pool-seq +
       transfer + sem-prop instead of the HWDGE + DGE-delay chain.
    2. The exit drain skips the DMASW completion lanes (the writeback only
       fires at the trigger); its receipt is waited at the very end on gpsimd
       so it overlaps the barrier and semaphore clears.
    3. The trailing all-engine barrier is dropped and PE excluded (single
       TileContext, PE unused).

    NRT re-executes a NEFF only after every engine halted, and gpsimd halts
    after the clears + receipt wait, so re-execution is safe."""

    out_dma_sem = None  # set by _build
    pre_sems = ()       # hand sems of pre-issued chunk-0 DMAs, cleared at exit

    def _drain_and_barrier(self, tick_clock, wait_clock):
        nc = self.nc
        gc = tick_clock.global_clock
        vec = [gc[p] for p in range(len(gc))]
        for p in _DMASW_PROCS:
            vec[p] = 0
        lean_clock = ScopedClock({None: VectorClock(vec)})
        # the stats writers run on ACT/DVE; the prep's desc-gen on Pool.
        # Waits on those three engine lanes imply everything else the
        # trigger needs (every DMA's completion precedes its consumer's
        # engine tick), and fewer wait ops cost less Pool-seq time.
        tvec = [0] * len(vec)
        for name in ("Activation", "DVE", "Pool"):
            p = PROC_NAME_TO_IDX[name]
            tvec[p] = vec[p]
        trig_clock = ScopedClock({None: VectorClock(tvec)})
        dma_sem = self.out_dma_sem
        # Fire the prepared stats writeback, gated on every engine's tile
        # clock (covers all stats writers — and, via the Pool lane, the
        # prep's descriptor generation; the stats buffer itself is a raw
        # SBUF tensor outside Tile tracking).  count=1 is rejected by this
        # runtime; count=None fires all pending preps.
        trig = nc.gpsimd.trigger_dma(count=None)
        wait_clock.add_sem_waits(trig.ins, trig_clock)
        drain_inst = nc.sync.drain()
        wait_clock.add_sem_waits(drain_inst.ins, lean_clock)
        nc.multi_engine_barrier(
            [
                mybir.EngineType.SP,
                mybir.EngineType.Activation,
                mybir.EngineType.DVE,
                mybir.EngineType.Pool,
            ]
        )
        popped = nc._tile_sem_poison_stack.pop()
        assert popped is self._sem_poison
        nc.clear_and_free_semaphores(list(self.sems.allocated().values()))
        nc.gpsimd.wait_ge(dma_sem, 16)
        nc.gpsimd.sem_clear(dma_sem)
        for s in self.pre_sems:
            nc.gpsimd.sem_clear(s)


def _chunk_list():
    """[(b, k, off, ch)] in processing order."""
    for b, plan in enumerate(PLANS):
        assert sum(plan) == F, f"plan {b} sums to {sum(plan)} != {F}"
    out = []
    k = 0
    for b, plan in enumerate(PLANS):
        off = 0
        for ch in plan:
            out.append((b, k, off, ch))
            off += ch
            k += 1
    return out


def _make_nc():
    """Bacc() materializes four [128,1] const APs via gpsimd memsets in the
    prologue, serialized on Pool before the prologue barrier (~95ns engine +
    61ns seq each).  This kernel only ever reads const fp32 0.0 (the sigmoid
    bias); skip the other three so the first input DMA issues earlier.

    The prologue all-engine barrier also excludes SP: it fences the const
    memsets and entry semaphore clears against use, but SP's first
    instructions are input DMAs that wait on nothing, increment semaphores
    only microseconds later (HWDGE + DGE latency + transfer), and every
    semaphore they touch was already cleared by the previous execution's
    exit sequence — so SP can start issuing immediately."""
    import concourse.bass as bassmod

    cls = bassmod.BassEitherVectorEngine
    orig_memset = cls.memset
    skip = ("const-float32-1.0", "const-bfloat16-1.0", "const-uint8-127")

    def patched_memset(self, ap, constant):
        name = getattr(getattr(ap, "tensor", None), "name", "") or ""
        if any(s in name for s in skip):
            return None
        return orig_memset(self, ap, constant)

    orig_aeb = bacc.Bacc.all_engine_barrier

    def patched_aeb(self, *, sem_only: bool = False):
        self.multi_engine_barrier(
            [e for e in self.engines if e != mybir.EngineType.SP]
        )

    cls.memset = patched_memset
    bacc.Bacc.all_engine_barrier = patched_aeb
    try:
        return bacc.Bacc("TRN2")
    finally:
        cls.memset = orig_memset
        bacc.Bacc.all_engine_barrier = orig_aeb


def _build(repeat=1):
    nc = _make_nc()
    pred = nc.dram_tensor("pred", [B, P, F], FP32, kind="ExternalInput")
    targ = nc.dram_tensor("target", [B, P, F], FP32, kind="ExternalInput")
    # out[0, :, 0, q*NCOLS + k]: q=0 -> sum sigmoid(p), q=1 -> sum p*t,
    # q=2 -> sum t.  4-D shape matches kv_writeback's
    # [batch, d_head_inner, d_head_outer, n_ctx] output contract.
    out = nc.dram_tensor("out", [1, P, 1, 3 * NCOLS], FP32, kind="ExternalOutput")

    dma_sem = nc.alloc_semaphore("out_dma_sem")

    chunks = _chunk_list()
    n_chunks = len(chunks)

    # Stats + writeback-index buffers live OUTSIDE Tile tracking: the
    # prepare-only writeback below must not acquire data deps on the stats
    # writers (Tile would gate the writers on the DMA's completion — a
    # cycle); ordering is enforced by hand at the trigger instead.
    st_cm = nc.sbuf_tensor("st", [P, 1, 1, 3 * NCOLS], FP32)
    ctx_cm = nc.sbuf_tensor("ctx", [P, 1], INT32)
    st_h = st_cm.__enter__()
    ctx_h = ctx_cm.__enter__()
    cms = [ctx_cm, st_cm]
    pre_dma_names = []
    pre_sems = []
    pre_bufs = {}
    if PRE_ISSUE:
        b0, k0, off0, ch0 = chunks[0]
        p0_cm = nc.sbuf_tensor("p0buf", [P, ch0], FP32)
        t0_cm = nc.sbuf_tensor("t0buf", [P, ch0], FP32)
        p0_h = p0_cm.__enter__()
        t0_h = t0_cm.__enter__()
        cms = [t0_cm, p0_cm] + cms
        p0_sem = nc.alloc_semaphore("p0_sem")
        t0_sem = nc.alloc_semaphore("t0_sem")
        pre_sems = [p0_sem, t0_sem]
        d0 = nc.sync.dma_start(out=p0_h[:, :], in_=pred[b0, :, 0:ch0])
        d0.then_inc(p0_sem, 16)
        d1 = nc.sync.dma_start(out=t0_h[:, :], in_=targ[b0, :, 0:ch0])
        d1.then_inc(t0_sem, 16)
        pre_dma_names = [d0.ins.name, d1.ins.name]
        pre_bufs = {"p": p0_h[:, :], "t": t0_h[:, :]}
    st = st_h[:, :, :, :]
    ctx = ctx_h[:, :]
    st_p = st_h[:, 0, 0, 0:NCOLS]
    st_pt = st_h[:, 0, 0, NCOLS : 2 * NCOLS]
    st_t = st_h[:, 0, 0, 2 * NCOLS : 3 * NCOLS]

    with _LeanTileContext(nc) as tc:
        tc.out_dma_sem = dma_sem
        tc.pre_sems = pre_sems
        with (
            tc.tile_pool(name="io", bufs=IO_BUFS) as io,
            tc.tile_pool(name="tmp", bufs=TMP_BUFS) as tmp,
        ):
            # descriptor prep on the idle Pool engine at kernel start; the
            # exit merely triggers it.  ctx memset precedes the prep in Pool
            # program order (the prep's ucode reads it).
            nc.gpsimd.memset(ctx, 0)
            prep = nc.gpsimd.kv_writeback(
                out[:, :, :, :],
                st,
                ctx,
                prepare_only=True,
                sem=dma_sem,
            )
            prep_name = prep.ins.name

            def _unfence(inst):
                # The prep only reads st's *addresses* at desc-gen time (the
                # data is read when the exit trigger fires, which hand-waits
                # on every engine's tile clock), so the WAR edge the overlap
                # tracker adds from each stats writer to the prep — whose
                # completion signal is the writeback DMA itself — must go, or
                # the first writer deadlocks against its own output DMA.
                # Deps on the pre-issued (out-of-context, tick-less) chunk-0
                # DMAs are likewise stripped; explicit wait_ge's stand in.
                inst.ins.try_remove_dependency(prep_name)
                for n in pre_dma_names:
                    inst.ins.try_remove_dependency(n)
                return inst

            for _ in range(repeat):
                # --- allocate tiles and issue DMAs in explicit transfer
                # order on the single SP HWDGE ring.  Body chunks issue
                # [p_k, t_k] pairs; the last TAIL chunks issue all their p's
                # first (so sigmoids complete early), then their t's, so the
                # final transfers feed only short DVE ops.
                p_tiles, t_tiles = {}, {}
                for b, k, off, ch in chunks:
                    if k == 0 and PRE_ISSUE:
                        p_tiles[k] = pre_bufs["p"]
                        t_tiles[k] = pre_bufs["t"]
                        continue
                    p_tiles[k] = io.tile([P, MAXC], FP32, tag="p_in", name=f"p_in{k}")
                    t_tiles[k] = io.tile([P, MAXC], FP32, tag="t_in", name=f"t_in{k}")

                body = chunks[: n_chunks - TAIL]
                tail = chunks[n_chunks - TAIL :]
                dma_order = []
                for b, k, off, ch in body:
                    dma_order.append(("p", b, k, off, ch))
                    dma_order.append(("t", b, k, off, ch))
                for b, k, off, ch in tail:
                    dma_order.append(("p", b, k, off, ch))
                for b, k, off, ch in tail:
                    dma_order.append(("t", b, k, off, ch))
                for which, b, k, off, ch in dma_order:
                    if k == 0 and PRE_ISSUE:
                        continue  # issued before context entry
                    cols = slice(off, off + ch)
                    if which == "p":
                        nc.sync.dma_start(
                            out=p_tiles[k][:, :ch], in_=pred[b, :, cols]
                        )
                    else:
                        nc.sync.dma_start(
                            out=t_tiles[k][:, :ch], in_=targ[b, :, cols]
                        )

                # --- compute, in chunk order
                for b, k, off, ch in chunks:
                    # tail chunks (except the very last) put sum(t) on ACT:
                    # their t's land in the final ~1.2us of the stream and
                    # DVE alone can't drain stt+sum(t) for all of them; ACT
                    # is idle once the prefetched tail sigmoids are done.
                    sumt = SUMT
                    if _TCOPY_IDX is not None:
                        if k in _TCOPY_IDX:
                            sumt = "tcopy"
                    elif n_chunks - TAIL <= k < n_chunks - 1 and ch in _TCOPY_WIDTHS:
                        sumt = "tcopy"
                    p_in, t_in = p_tiles[k], t_tiles[k]
                    if k == 0 and PRE_ISSUE:
                        nc.scalar.wait_ge(pre_sems[0], 16)
                        nc.vector.wait_ge(pre_sems[1], 16)
                    sig = tmp.tile([P, MAXC], FP32, tag="sig")
                    _unfence(nc.scalar.activation(
                        sig[:, :ch],
                        p_in[:, :ch],
                        mybir.ActivationFunctionType.Sigmoid,
                        accum_out=st_p[:, k : k + 1],
                    ))
                    # sum(t) first: it needs only t, so DVE starts it while
                    # the product below still waits on the sigmoid's sem.
                    # prod/tout are discarded side-outputs of the fused
                    # accumulate ops: bf16 halves their SBUF write traffic
                    if sumt == "ts":
                        tout = tmp.tile([P, MAXC], BF16, tag="tout")
                        _unfence(nc.vector.tensor_scalar(
                            out=tout[:, :ch],
                            in0=t_in[:, :ch],
                            scalar1=1.0,
                            scalar2=0.0,
                            op0=mybir.AluOpType.mult,
                            op1=mybir.AluOpType.add,
                            accum_out=st_t[:, k : k + 1],
                        ))
                    elif sumt == "reduce":
                        _unfence(nc.vector.tensor_reduce(
                            out=st_t[:, k : k + 1],
                            in_=t_in[:, :ch],
                            axis=mybir.AxisListType.X,
                            op=mybir.AluOpType.add,
                        ))
                    else:  # tcopy on ACT
                        tout = tmp.tile([P, MAXC], BF16, tag="tout")
                        _unfence(nc.scalar.activation(
                            tout[:, :ch],
                            t_in[:, :ch],
                            mybir.ActivationFunctionType.Copy,
                            accum_out=st_t[:, k : k + 1],
                        ))
                    prod = tmp.tile([P, MAXC], BF16, tag="prod")
                    _unfence(nc.vector.scalar_tensor_tensor(
                        out=prod[:, :ch],
                        in0=sig[:, :ch],
                        scalar=0.0,
                        in1=t_in[:, :ch],
                        op0=mybir.AluOpType.bypass,
                        op1=mybir.AluOpType.mult,
                        accum_out=st_pt[:, k : k + 1],
                    ))

    for cm in cms:
        cm.__exit__(None, None, None)
    nc.compile()
    return nc


def run(pred, target, weight, **spmd_kwargs):
    global _nc_cache
    if _nc_cache is None:
        _nc_cache = _build()
    nc = _nc_cache

    p2 = np.asarray(pred, dtype=np.float32).reshape(B, N)
    t2 = np.asarray(target, dtype=np.float32).reshape(B, N)
    in_maps = []
    for i in range(N_CORES):
        sl = slice(i * SHARD, (i + 1) * SHARD)
        in_maps.append(
            {
                "pred": np.ascontiguousarray(p2[:, sl]).reshape(B, P, F),
                "target": np.ascontiguousarray(t2[:, sl]).reshape(B, P, F),
            }
        )
    res = run_bass_kernel_spmd(nc, in_maps, core_ids=list(range(N_CORES)), **spmd_kwargs)

    partials = np.stack(
        [np.asarray(r["out"]).reshape(P, 3 * NCOLS) for r in res.results]
    )  # [8, P, 3*NCOLS]
    grp = partials.reshape(N_CORES, P, 3, NCOLS)
    # per-sample sums over cores, partitions, and that sample's chunk columns
    s_b = np.empty((3, B), dtype=np.float64)
    for b in range(B):
        lo, hi = SAMPLE_COL_OFFSETS[b], SAMPLE_COL_OFFSETS[b + 1]
        s_b[:, b] = grp[:, :, :, lo:hi].sum(axis=(0, 1, 3), dtype=np.float64)
    psum, inter, tsum = s_b[0], s_b[1], s_b[2]
    w = np.asarray(weight, dtype=np.float64)
    smooth = 1.0
    dice = (2.0 * inter * w + smooth) / (psum * w + tsum * w + smooth)
    loss = np.sum(1.0 - dice) / B
    return np.array(loss, dtype=np.float32), res


def kernel(pred, target, weight):
    loss, _ = run(pred, target, weight)
    return loss
